# revision 36
# baseline (speedup 1.0000x reference)
"""Multi-head causal attention (B=4, S=2048, E=1024, H=16, D=64) on 8 TRN2 cores.

Sharding: core c handles batch c//2 and head-half c%2 (8 heads). Each core
computes Q/K/V projections, causal attention for its heads, and a partial
output projection over its heads. Partials are exchanged per 256-token
chunk with a bf16 ReduceScatter over the core pair; the RS splits the
embedding dim, so each core stores one E-half of the final output and the
host transposes/concatenates.

Layouts are transposed (feature-on-partition): the host supplies X^T and
head-packed weights so no on-chip transposes are needed. All inputs are
packed [128, k, n] so each tensor loads with one or two large DMAs.
Attention runs in S^T = K.Q^T layout (keys on partitions); softmax
denominators come from a ones-column appended to V so the PV matmul emits
them for free. Scores for a head pair go side by side into one 2-bank
PSUM tile so a single ScalarE exp covers both; diagonal blocks skip the
causally dead columns in the matmuls, the exp and the mask multiply.

The output projection is emitted transposed (Wo chunk stationary, ao
moving) so the bias lands as a per-partition tensor_scalar on the
PSUM->SBUF copy instead of a K=1 matmul, and the two softmax reciprocal
broadcasts of a head pair share one K=2 matmul.

Scheduling: the exp (ScalarE) paces the attention inner loop, so the PE
needs independent work wherever it would wait. Q/K projections for q-tiles
1-3 and V projections for key-tiles 4-15 are deferred into 8-matmul filler
units pulled between the scores and PV matmuls of the attention loop, and
output-projection units for the previous q-tile run between each pair's
attention and its normalization. This keeps the PE dense enough that the
HAM clock gate stays at full rate.
"""

import sys

sys.path.insert(0, "/opt/trn_rl_repo")

from collections import deque

import numpy as np
import ml_dtypes

import concourse.bass as bass
import concourse.bacc as bacc
import concourse.tile as tile
import concourse.mybir as mybir
import concourse.bass_utils as bass_utils

B, S, E, H, D = 4, 2048, 1024, 16, 64
N_CORES = 8
HPC = H // 2          # heads per core
NPAIR = HPC // 2      # head pairs per core
SQ = 512              # q tile width
SK = 128              # k tile width
NQT = S // SQ         # 4
NKT = S // SK         # 16
NE = E // 128         # 8 contraction tiles
NCH = 8               # output exchange chunks (256 tokens each)
CHT = S // NCH        # 256 tokens per chunk
F32 = mybir.dt.float32
F32R = mybir.dt.float32r
BF16 = mybir.dt.bfloat16
BF16_NP = ml_dtypes.bfloat16

REPLICA_GROUPS = [[0, 1], [2, 3], [4, 5], [6, 7]]
AF = mybir.ActivationFunctionType
ALU = mybir.AluOpType


def build_kernel():
    nc = bacc.Bacc("TRN2", target_bir_lowering=False, debug=False,
                   num_devices=N_CORES)

    xta_d = nc.dram_tensor("XTa", [128, NE, SQ], BF16, kind="ExternalInput")
    xtb_d = nc.dram_tensor("XTb", [128, NE, S - SQ], BF16, kind="ExternalInput")
    wq_d = nc.dram_tensor("Wq", [128, NE, HPC * D], BF16, kind="ExternalInput")
    wk_d = nc.dram_tensor("Wk", [128, NE, HPC * D], BF16, kind="ExternalInput")
    wv_d = nc.dram_tensor("Wv", [128, NE, HPC * D], BF16, kind="ExternalInput")
    wo_d = nc.dram_tensor("Wo", [128, NPAIR, E], BF16, kind="ExternalInput")
    cst_d = nc.dram_tensor("cst", [128, 2 * NPAIR + NE], F32, kind="ExternalInput")
    ones_d = nc.dram_tensor("ones", [1, 128], F32R, kind="ExternalInput")
    mask_d = nc.dram_tensor("masks", [SK, 4, 2 * SQ], BF16, kind="ExternalInput")
    # each core stores its E-half of the output, transposed: [512, S]
    out_d = nc.dram_tensor("out", [E // 2, S], BF16, kind="ExternalOutput")

    with tile.TileContext(nc) as tc:
        with (
            tc.tile_pool(name="persist", bufs=1) as persist,
            tc.tile_pool(name="dram", bufs=1, space="DRAM") as dram,
            tc.tile_pool(name="p1_in", bufs=1) as p1_in,
            tc.tile_pool(name="mm_ps", bufs=2, space="PSUM") as mm_ps,
            tc.tile_pool(name="st_ps", bufs=2, space="PSUM") as st_ps,
            tc.tile_pool(name="pv_ps", bufs=1, space="PSUM") as pv_ps,
            tc.tile_pool(name="probs", bufs=4) as probs_pool,
            tc.tile_pool(name="norm", bufs=2) as norm_pool,
            tc.tile_pool(name="op_sb", bufs=2) as op_sb,
        ):
            # ---- persistent SBUF tensors ----
            qt_sb = [persist.tile([128, S], BF16, tag=f"qt{p}", name=f"qt{p}")
                     for p in range(NPAIR)]
            kt_sb = [persist.tile([128, S], BF16, tag=f"kt{p}", name=f"kt{p}")
                     for p in range(NPAIR)]
            v_sb = [persist.tile([128, HPC, D + 1], BF16, tag=f"v{st}", name=f"v{st}")
                    for st in range(NKT)]
            # ao2[c][:, i, :] holds pair 2c+i (partition = dim within pair)
            ao_sb = [persist.tile([128, 2, S], BF16, tag=f"ao{c}", name=f"ao{c}")
                     for c in range(NPAIR // 2)]
            wo_sb = persist.tile([128, NPAIR, E], BF16, tag="wo")
            cst_sb = persist.tile([128, 2 * NPAIR + NE], F32, tag="cst")
            ones_sb = persist.tile([1, 128], F32R, tag="ones")
            mask_sb = persist.tile([SK, 4, 2 * SQ], BF16, tag="mask")
            et_sb = persist.tile([SK, 4, 2 * SQ], BF16, tag="et")
            cc_in = [dram.tile([E, CHT], BF16, name=f"cc_in{c}")
                     for c in range(NCH)]
            cc_out = [dram.tile([E // 2, CHT], BF16, name=f"cc_out{c}")
                      for c in range(NCH)]

            xta_sb = p1_in.tile([128, NE, SQ], BF16, tag="xta")
            xtb_sb = p1_in.tile([128, NE, S - SQ], BF16, tag="xtb")
            wq_sb = p1_in.tile([128, NE, HPC * D], BF16, tag="wq")
            wk_sb = p1_in.tile([128, NE, HPC * D], BF16, tag="wk")
            wv_sb = p1_in.tile([128, NE, HPC * D], BF16, tag="wv")
            # DMA order = consumption order. Consolidated: one or two large
            # descriptors per tensor so the sync queue issues ~14 DMAs
            # total; the first matmul (V proj) waits only on wv + xt cols
            # 0-511, so those go first and the tiny constants follow the
            # big pre-attention tensors.
            nc.sync.dma_start(wv_sb[:], wv_d[:])
            nc.sync.dma_start(xta_sb[:], xta_d[:])
            nc.sync.dma_start(wq_sb[:], wq_d[:])
            nc.sync.dma_start(wk_sb[:], wk_d[:])
            nc.sync.dma_start(mask_sb[:, 0, :], mask_d[:, 0, :])
            nc.sync.dma_start(cst_sb[:], cst_d[:])
            nc.sync.dma_start(ones_sb[:], ones_d[:])
            for oi in range(1, 4):
                nc.sync.dma_start(mask_sb[:, oi, :], mask_d[:, oi, :])
            nc.sync.dma_start(xtb_sb[:], xtb_d[:])
            nc.sync.dma_start(wo_sb[:], wo_d[:])

            def xt_cols(e, lo, hi):
                if hi <= SQ:
                    return xta_sb[:, e, lo:hi]
                return xtb_sb[:, e, lo - SQ:hi - SQ]

            def emit_v(st):
                ps = mm_ps.tile([128, HPC * D], F32, tag="mm", name="psv")
                for e in range(NE):
                    nc.tensor.matmul(
                        ps[:],
                        xt_cols(e, st * 128, (st + 1) * 128),
                        wv_sb[:, e, :],
                        start=(e == 0), stop=(e == NE - 1),
                    )
                nc.vector.tensor_copy(
                    v_sb[st][:, :, 0:D],
                    ps[:].rearrange("p (h d) -> p h d", h=HPC),
                )
                nc.vector.memset(v_sb[st][:, :, D:D + 1], 1.0)

            def emit_qk_unit(p, st, which):
                w_sb, o_sb = ((wq_sb, qt_sb), (wk_sb, kt_sb))[which]
                ps = mm_ps.tile([128, SQ], F32, tag="mm", name="ps")
                for e in range(NE):
                    nc.tensor.matmul(
                        ps[:],
                        w_sb[:, e, p * 128:(p + 1) * 128],
                        xt_cols(e, st * SQ, (st + 1) * SQ),
                        start=(e == 0), stop=(e == NE - 1),
                    )
                nc.vector.tensor_scalar(
                    o_sb[p][:, st * SQ:(st + 1) * SQ], ps[:],
                    cst_sb[:, which * NPAIR + p:which * NPAIR + p + 1],
                    None, ALU.add,
                )

            # ---- deferred projection units, pulled as PE filler ----
            # unit (deadline, kind, args): deadline = attention step 4*qt+p
            # by which the unit's output is first consumed
            filler = []
            for st in range(4, NKT):
                filler.append((4 * (st // 4), 'v', st))
            for p in range(1, NPAIR):
                for w in range(2):
                    filler.append((p, 'qk', p, 0, w))
            for st in range(1, NQT):
                for p in range(NPAIR):
                    for w in range(2):
                        filler.append((4 * st + p, 'qk', p, st, w))
            filler.sort(key=lambda t: t[0])
            fq = deque(filler)

            def emit_unit(u):
                if u[1] == 'v':
                    emit_v(u[2])
                else:
                    emit_qk_unit(u[2], u[3], u[4])

            def flush_due(s):
                while fq and fq[0][0] <= s:
                    emit_unit(fq.popleft())

            def pull_filler():
                if fq:
                    emit_unit(fq.popleft())

            def attn_kt(qt, p, kt, pv, emit_pv=True):
                nkt_q = 4 * (qt + 1)
                ks = slice(kt * SK, (kt + 1) * SK)
                oi = kt - 4 * qt
                vq = oi * SK if oi > 0 else 0
                sps = st_ps.tile([128, 2 * SQ], F32, tag="st", name="st")
                for u in range(2):
                    nc.tensor.matmul(
                        sps[:, u * SQ + vq:(u + 1) * SQ],
                        kt_sb[p][u * D:(u + 1) * D, ks],
                        qt_sb[p][u * D:(u + 1) * D,
                                 qt * SQ + vq:(qt + 1) * SQ],
                        start=True, stop=True,
                    )
                pt = probs_pool.tile([128, 2 * SQ], BF16, tag="pt", name="pt")
                if oi >= 0:  # diagonal block: exp + mask the valid columns
                    sps3 = sps[:].rearrange("p (u q) -> p u q", u=2)
                    et3 = et_sb[:, oi, :].rearrange("p (u q) -> p u q", u=2)
                    pt3 = pt[:].rearrange("p (u q) -> p u q", u=2)
                    m3 = mask_sb[:, oi, :].rearrange("p (u q) -> p u q", u=2)
                    nc.scalar.activation(et3[:, :, vq:], sps3[:, :, vq:],
                                         AF.Exp, scale=0.125)
                    nc.vector.tensor_tensor(pt3[:, :, vq:], et3[:, :, vq:],
                                            m3[:, :, vq:], ALU.mult)
                else:
                    nc.scalar.activation(pt[:], sps[:], AF.Exp, scale=0.125)
                # keep the PE fed while the exp runs: the filler sits
                # between the scores and PV matmuls in the PE queue;
                # spend units sparingly early so qt3 (most key tiles,
                # no other filler) does not run dry
                if kt % (4 if qt < 3 else 2) == 1:
                    pull_filler()
                if not emit_pv:
                    return pt, vq
                for u in range(2):
                    nc.tensor.matmul(
                        pv[u][0:D + 1, vq:],
                        v_sb[kt][:, 2 * p + u, :],
                        pt[:, u * SQ + vq:(u + 1) * SQ],
                        start=(kt == 0), stop=(kt == nkt_q - 1),
                    )
                return None

            def attn_pv(qt, p, kt, pv, pt, vq):
                nkt_q = 4 * (qt + 1)
                for u in range(2):
                    nc.tensor.matmul(
                        pv[u][0:D + 1, vq:],
                        v_sb[kt][:, 2 * p + u, :],
                        pt[:, u * SQ + vq:(u + 1) * SQ],
                        start=(kt == 0), stop=(kt == nkt_q - 1),
                    )

            def attn_head(qt, p):
                # next pair's first two score+exp tiles, issued before the
                # previous pair's normalization so the exp latency hides
                # behind it (touches only the score/probs rings)
                pv = [pv_ps.tile([128, SQ], F32, tag=f"pv{u}", name=f"pv{u}")
                      for u in range(2)]
                heads = [(kt,) + attn_kt(qt, p, kt, pv, emit_pv=False)
                         for kt in (0, 1)]
                return pv, heads

            def attn_rest(qt, p, pv, heads):
                nkt_q = 4 * (qt + 1)
                for kt, pt, vq in heads:
                    attn_pv(qt, p, kt, pv, pt, vq)
                for kt in range(2, nkt_q):
                    attn_kt(qt, p, kt, pv)
                return pv

            def attn_norm(qt, p, pv):
                # den copies off partition 64 (the custom reciprocal needs a
                # partition-0 input), one reciprocal per head into a shared
                # row, ONE Pool-engine partition_broadcast for the pair, and
                # the multiplies read pv straight from PSUM (the broadcast
                # lives in SBUF, so the one-PSUM-operand rule is satisfied
                # with no staging copies)
                qs = slice(qt * SQ, (qt + 1) * SQ)
                rcp2 = norm_pool.tile([1, 2, SQ], F32, tag="rcp2", name="rcp2")
                for u in range(2):
                    den = norm_pool.tile([1, SQ], F32, tag="den", name="den")
                    nc.vector.tensor_copy(den[:], pv[u][D:D + 1, :])
                    nc.vector.reciprocal_approx_fast(rcp2[:, u, :], den[:])
                rcpb = norm_pool.tile([D, 2, SQ], F32, tag="rcpb", name="rcpb")
                nc.gpsimd.partition_broadcast(rcpb[:], rcp2[:])
                for u in range(2):
                    nc.vector.tensor_tensor(
                        ao_sb[p // 2][u * D:(u + 1) * D, p % 2, qs],
                        pv[u][0:D, :], rcpb[:, u, :], ALU.mult,
                    )

            def emit_op_unit(cq, e, ob):
                # transposed output projection: out^T[e-chunk, tokens]
                ts = slice(cq * CHT, (cq + 1) * CHT)
                ps = mm_ps.tile([128, SQ], F32, tag="mm", name="op")
                for c in range(NPAIR // 2):
                    for i in range(2):
                        nc.tensor.matmul(
                            ps[:, 0:CHT],
                            wo_sb[:, 2 * c + i, e * 128:(e + 1) * 128],
                            ao_sb[c][:, i, ts],
                            start=(c == 0 and i == 0),
                            stop=(c == NPAIR // 2 - 1 and i == 1),
                        )
                nc.vector.tensor_scalar(
                    ob[:, e, :], ps[:, 0:CHT],
                    cst_sb[:, 2 * NPAIR + e:2 * NPAIR + e + 1], None, ALU.add,
                )

            def new_ob():
                return op_sb.tile([128, NE, CHT], BF16, tag="ob", name="ob")

            def emit_rs(cq):
                nc.gpsimd.collective_compute(
                    "ReduceScatter", ALU.add,
                    ins=[cc_in[cq][:].opt()],
                    outs=[cc_out[cq][:].opt()],
                    replica_groups=REPLICA_GROUPS,
                )
                nc.sync.dma_start(out_d[:, cq * CHT:(cq + 1) * CHT],
                                  cc_out[cq][:])

            # ---- schedule ----
            # output units per step: (chunk, [e-list]). Chunks lag their
            # q-tile by one step-group; qt=3's steps pull chunks 4 and 5
            # forward so only chunks 6,7 remain for the tail.
            op_sched = {}
            for qt in (1, 2):
                c0, c1 = 2 * (qt - 1), 2 * (qt - 1) + 1
                op_sched[4 * qt + 0] = [(c0, range(0, 4))]
                op_sched[4 * qt + 1] = [(c0, range(4, 8))]
                op_sched[4 * qt + 2] = [(c1, range(0, 4))]
                op_sched[4 * qt + 3] = [(c1, range(4, 8))]
            op_sched[12] = [(4, range(0, 6))]
            op_sched[13] = [(4, range(6, 8)), (5, range(0, 4))]
            op_sched[14] = [(5, range(4, 6))]
            op_sched[15] = [(5, range(6, 8))]

            for st in range(4):
                emit_v(st)
            emit_qk_unit(0, 0, 0)
            emit_qk_unit(0, 0, 1)
            steps = [(qt, p) for qt in range(NQT) for p in range(NPAIR)]
            nxt = attn_head(0, 0)
            obs = {}
            for qt in range(NQT):
                for p in range(NPAIR):
                    s = 4 * qt + p
                    flush_due(s)
                    pv = attn_rest(qt, p, *nxt)
                    # next step's just-in-time units double as filler over
                    # this pair's normalization chain; its first score+exp
                    # tiles go out before the norm too
                    flush_due(s + 1)
                    nxt = (attn_head(*steps[s + 1])
                           if s + 1 < len(steps) else None)
                    last = s + 1 == len(steps)
                    if last:
                        # norm is DVE/Pool-only; issuing it first lets its
                        # chain overlap the op units and the tail prestart
                        attn_norm(qt, p, pv)
                    for cq, es in op_sched.get(s, []):
                        if cq not in obs:
                            obs[cq] = new_ob()
                        for e in es:
                            emit_op_unit(cq, e, obs[cq])
                        if es[-1] == 7:
                            nc.sync.dma_start(
                                cc_in[cq][:].rearrange("(e p) t -> p e t",
                                                       p=128), obs[cq][:])
                            emit_rs(cq)
                    if not last:
                        attn_norm(qt, p, pv)
            # tail: the last q-tile's two chunks; per-unit DMAs so each RS
            # fires right after its last unit's copy. The first chunk's
            # units pre-start their pair 0-2 matmuls across the attention's
            # now-idle PSUM rings so only the pair-3 matmuls wait for the
            # final pair's norm.
            def op_partial(cq, e, ps, pairs):
                ts = slice(cq * CHT, (cq + 1) * CHT)
                for c, i in pairs:
                    nc.tensor.matmul(
                        ps[:, 0:CHT],
                        wo_sb[:, 2 * c + i, e * 128:(e + 1) * 128],
                        ao_sb[c][:, i, ts],
                        start=(c == 0 and i == 0),
                        stop=(c == 1 and i == 1),
                    )

            cq0 = 2 * (NQT - 1)
            ob = new_ob()
            pre = ([(mm_ps, "mm", [128, SQ])] * 2
                   + [(st_ps, "st", [128, 2 * SQ])] * 2
                   + [(pv_ps, "pv0", [128, SQ]), (pv_ps, "pv1", [128, SQ])])
            pss = []
            for e in range(6):
                pool, tag, shape = pre[e]
                ps = pool.tile(shape, F32, tag=tag, name=f"tail{e}")
                op_partial(cq0, e, ps, [(0, 0), (0, 1), (1, 0)])
                pss.append(ps)
            for e in range(6):
                op_partial(cq0, e, pss[e], [(1, 1)])
                nc.vector.tensor_scalar(
                    ob[:, e, :], pss[e][:, 0:CHT],
                    cst_sb[:, 2 * NPAIR + e:2 * NPAIR + e + 1], None, ALU.add)
                nc.sync.dma_start(cc_in[cq0][e * 128:(e + 1) * 128, :],
                                  ob[:, e, :])
            for e in range(6, NE):
                emit_op_unit(cq0, e, ob)
                nc.sync.dma_start(cc_in[cq0][e * 128:(e + 1) * 128, :],
                                  ob[:, e, :])
            emit_rs(cq0)
            ob = new_ob()
            for e in range(NE):
                emit_op_unit(cq0 + 1, e, ob)
                nc.sync.dma_start(cc_in[cq0 + 1][e * 128:(e + 1) * 128, :],
                                  ob[:, e, :])
            emit_rs(cq0 + 1)

    nc.compile()
    return nc


_NC_CACHE = None


def get_nc():
    global _NC_CACHE
    if _NC_CACHE is None:
        _NC_CACHE = build_kernel()
    return _NC_CACHE


def make_in_maps(X, Wq, Wk, Wv, bq, bk, bv, Wo, bo):
    X = np.asarray(X, np.float32)
    Wq, Wk, Wv = (np.asarray(w, np.float32) for w in (Wq, Wk, Wv))
    bq, bk, bv = (np.asarray(b, np.float32) for b in (bq, bk, bv))
    Wo = np.asarray(Wo, np.float32)
    bo = np.asarray(bo, np.float32)

    m = (np.arange(SQ)[None, :] >=
         (np.arange(4)[:, None, None] * 128 + np.arange(SK)[None, :, None])
         ).astype(BF16_NP)                       # [4, 128, 512]
    masks = np.concatenate([m, m], axis=2)       # [4, 128, 1024] (head pair)
    masks = np.ascontiguousarray(masks.transpose(1, 0, 2))  # [128, 4, 1024]

    def pack(w):  # [1024, n] -> [128, 8, n]
        n = w.shape[1]
        return np.ascontiguousarray(
            w.reshape(NE, 128, n).transpose(1, 0, 2).astype(BF16_NP))

    in_maps = []
    for c in range(N_CORES):
        b, hh = c // 2, c % 2
        hs = slice(hh * HPC, (hh + 1) * HPC)
        wo_c = Wo[hh * HPC * D:(hh + 1) * HPC * D]          # [512, E]
        # fold V-bias and half the output bias into one bias column
        bo2 = 0.5 * bo + bv[hs].reshape(HPC * D) @ wo_c     # [1024]
        bqk = np.concatenate([bq[hs].reshape(NPAIR, 128),
                              bk[hs].reshape(NPAIR, 128)], axis=0)
        in_maps.append({
            "XTa": pack(X[b].T[:, 0:SQ]),
            "XTb": pack(X[b].T[:, SQ:]),
            "Wq": pack(Wq[hs].transpose(1, 0, 2).reshape(E, HPC * D)),
            "Wk": pack(Wk[hs].transpose(1, 0, 2).reshape(E, HPC * D)),
            "Wv": pack(Wv[hs].transpose(1, 0, 2).reshape(E, HPC * D)),
            "Wo": np.ascontiguousarray(
                wo_c.reshape(NPAIR, 128, E).transpose(1, 0, 2)
                .astype(BF16_NP)),                           # [128, 4, 1024]
            "cst": np.ascontiguousarray(
                np.concatenate([bqk, bo2.reshape(NE, 128)], axis=0).T),
            "ones": np.ones((1, 128), np.float32),
            "masks": masks,
        })
    return in_maps


def assemble_output(results):
    out = np.empty((B, S, E), np.float32)
    for b in range(B):
        ev = results[2 * b]["out"].astype(np.float32)      # [512, S] E 0-511
        od = results[2 * b + 1]["out"].astype(np.float32)  # [512, S] E 512+
        out[b, :, 0:E // 2] = ev.T
        out[b, :, E // 2:] = od.T
    return out


def run(in_maps, **kw):
    nc = get_nc()
    return bass_utils.run_bass_kernel_spmd(nc, in_maps,
                                           core_ids=list(range(N_CORES)), **kw)


def kernel(X, Wq, Wk, Wv, bq, bk, bv, Wo, bo):
    in_maps = make_in_maps(X, Wq, Wk, Wv, bq, bk, bv, Wo, bo)
    res = run(in_maps)
    return assemble_output(res.results)


# revision 37
# speedup vs baseline: 1.0075x; 1.0075x over previous
"""Multi-head causal attention (B=4, S=2048, E=1024, H=16, D=64) on 8 TRN2 cores.

Sharding: core c handles batch c//2 and head-half c%2 (8 heads). Each core
computes Q/K/V projections, causal attention for its heads, and a partial
output projection over its heads. Partials are exchanged per 256-token
chunk with a bf16 ReduceScatter over the core pair; the RS splits the
embedding dim, so each core stores one E-half of the final output and the
host transposes/concatenates.

Layouts are transposed (feature-on-partition): the host supplies X^T and
head-packed weights so no on-chip transposes are needed. All inputs are
packed [128, k, n] so each tensor loads with one or two large DMAs.
Attention runs in S^T = K.Q^T layout (keys on partitions); softmax
denominators come from a ones-column appended to V so the PV matmul emits
them for free. Scores for a head pair go side by side into one 2-bank
PSUM tile so a single ScalarE exp covers both; diagonal blocks skip the
causally dead columns in the matmuls, the exp and the mask multiply.

The output projection is emitted transposed (Wo chunk stationary, ao
moving) so the bias lands as a per-partition tensor_scalar on the
PSUM->SBUF copy instead of a K=1 matmul, and the two softmax reciprocal
broadcasts of a head pair share one K=2 matmul.

Scheduling: the exp (ScalarE) paces the attention inner loop, so the PE
needs independent work wherever it would wait. Q/K projections for q-tiles
1-3 and V projections for key-tiles 4-15 are deferred into 8-matmul filler
units pulled between the scores and PV matmuls of the attention loop, and
output-projection units for the previous q-tile run between each pair's
attention and its normalization. This keeps the PE dense enough that the
HAM clock gate stays at full rate.
"""

import sys

sys.path.insert(0, "/opt/trn_rl_repo")

from collections import deque

import numpy as np
import ml_dtypes

import concourse.bass as bass
import concourse.bacc as bacc
import concourse.tile as tile
import concourse.mybir as mybir
import concourse.bass_utils as bass_utils

B, S, E, H, D = 4, 2048, 1024, 16, 64
N_CORES = 8
HPC = H // 2          # heads per core
NPAIR = HPC // 2      # head pairs per core
SQ = 512              # q tile width
SK = 128              # k tile width
NQT = S // SQ         # 4
NKT = S // SK         # 16
NE = E // 128         # 8 contraction tiles
NCH = 8               # output exchange chunks (256 tokens each)
CHT = S // NCH        # 256 tokens per chunk
F32 = mybir.dt.float32
F32R = mybir.dt.float32r
BF16 = mybir.dt.bfloat16
BF16_NP = ml_dtypes.bfloat16

REPLICA_GROUPS = [[0, 1], [2, 3], [4, 5], [6, 7]]
AF = mybir.ActivationFunctionType
ALU = mybir.AluOpType


def build_kernel():
    nc = bacc.Bacc("TRN2", target_bir_lowering=False, debug=False,
                   num_devices=N_CORES)

    xta_d = nc.dram_tensor("XTa", [128, NE, SQ], BF16, kind="ExternalInput")
    xtb_d = nc.dram_tensor("XTb", [128, NE, S - SQ], BF16, kind="ExternalInput")
    wq_d = nc.dram_tensor("Wq", [128, NE, HPC * D], BF16, kind="ExternalInput")
    wk_d = nc.dram_tensor("Wk", [128, NE, HPC * D], BF16, kind="ExternalInput")
    wv_d = nc.dram_tensor("Wv", [128, NE, HPC * D], BF16, kind="ExternalInput")
    wo_d = nc.dram_tensor("Wo", [128, NPAIR, E], BF16, kind="ExternalInput")
    cst_d = nc.dram_tensor("cst", [128, 2 * NPAIR + NE], F32, kind="ExternalInput")
    ones_d = nc.dram_tensor("ones", [1, 128], F32R, kind="ExternalInput")
    mask_d = nc.dram_tensor("masks", [SK, 4, 2 * SQ], BF16, kind="ExternalInput")
    # each core stores its E-half of the output, transposed: [512, S]
    out_d = nc.dram_tensor("out", [E // 2, S], BF16, kind="ExternalOutput")

    with tile.TileContext(nc) as tc:
        with (
            tc.tile_pool(name="persist", bufs=1) as persist,
            tc.tile_pool(name="dram", bufs=1, space="DRAM") as dram,
            tc.tile_pool(name="p1_in", bufs=1) as p1_in,
            tc.tile_pool(name="mm_ps", bufs=2, space="PSUM") as mm_ps,
            tc.tile_pool(name="st_ps", bufs=2, space="PSUM") as st_ps,
            tc.tile_pool(name="pv_ps", bufs=1, space="PSUM") as pv_ps,
            tc.tile_pool(name="probs", bufs=4) as probs_pool,
            tc.tile_pool(name="norm", bufs=2) as norm_pool,
            tc.tile_pool(name="op_sb", bufs=2) as op_sb,
        ):
            # ---- persistent SBUF tensors ----
            qt_sb = [persist.tile([128, S], BF16, tag=f"qt{p}", name=f"qt{p}")
                     for p in range(NPAIR)]
            kt_sb = [persist.tile([128, S], BF16, tag=f"kt{p}", name=f"kt{p}")
                     for p in range(NPAIR)]
            v_sb = [persist.tile([128, HPC, D + 1], BF16, tag=f"v{st}", name=f"v{st}")
                    for st in range(NKT)]
            # ao2[c][:, i, :] holds pair 2c+i (partition = dim within pair)
            ao_sb = [persist.tile([128, 2, S], BF16, tag=f"ao{c}", name=f"ao{c}")
                     for c in range(NPAIR // 2)]
            wo_sb = persist.tile([128, NPAIR, E], BF16, tag="wo")
            cst_sb = persist.tile([128, 2 * NPAIR + NE], F32, tag="cst")
            ones_sb = persist.tile([1, 128], F32R, tag="ones")
            mask_sb = persist.tile([SK, 4, 2 * SQ], BF16, tag="mask")
            et_sb = persist.tile([SK, 4, 2 * SQ], BF16, tag="et")
            cc_in = [dram.tile([E, CHT], BF16, name=f"cc_in{c}")
                     for c in range(NCH - 2)]
            cc_in.append(dram.tile([E, 2 * CHT], BF16, name="cc_in_tail"))
            cc_out = [dram.tile([E // 2, CHT], BF16, name=f"cc_out{c}")
                      for c in range(NCH - 2)]
            cc_out.append(dram.tile([E // 2, 2 * CHT], BF16,
                                    name="cc_out_tail"))

            xta_sb = p1_in.tile([128, NE, SQ], BF16, tag="xta")
            xtb_sb = p1_in.tile([128, NE, S - SQ], BF16, tag="xtb")
            wq_sb = p1_in.tile([128, NE, HPC * D], BF16, tag="wq")
            wk_sb = p1_in.tile([128, NE, HPC * D], BF16, tag="wk")
            wv_sb = p1_in.tile([128, NE, HPC * D], BF16, tag="wv")
            # DMA order = consumption order. Consolidated: one or two large
            # descriptors per tensor so the sync queue issues ~14 DMAs
            # total; the first matmul (V proj) waits only on wv + xt cols
            # 0-511, so those go first and the tiny constants follow the
            # big pre-attention tensors.
            nc.sync.dma_start(wv_sb[:], wv_d[:])
            nc.sync.dma_start(xta_sb[:], xta_d[:])
            nc.sync.dma_start(wq_sb[:], wq_d[:])
            nc.sync.dma_start(wk_sb[:], wk_d[:])
            nc.sync.dma_start(mask_sb[:, 0, :], mask_d[:, 0, :])
            nc.sync.dma_start(cst_sb[:], cst_d[:])
            nc.sync.dma_start(ones_sb[:], ones_d[:])
            for oi in range(1, 4):
                nc.sync.dma_start(mask_sb[:, oi, :], mask_d[:, oi, :])
            nc.sync.dma_start(xtb_sb[:], xtb_d[:])
            nc.sync.dma_start(wo_sb[:], wo_d[:])

            def xt_cols(e, lo, hi):
                if hi <= SQ:
                    return xta_sb[:, e, lo:hi]
                return xtb_sb[:, e, lo - SQ:hi - SQ]

            def emit_v(st):
                ps = mm_ps.tile([128, HPC * D], F32, tag="mm", name="psv")
                for e in range(NE):
                    nc.tensor.matmul(
                        ps[:],
                        xt_cols(e, st * 128, (st + 1) * 128),
                        wv_sb[:, e, :],
                        start=(e == 0), stop=(e == NE - 1),
                    )
                nc.vector.tensor_copy(
                    v_sb[st][:, :, 0:D],
                    ps[:].rearrange("p (h d) -> p h d", h=HPC),
                )
                nc.vector.memset(v_sb[st][:, :, D:D + 1], 1.0)

            def emit_qk_unit(p, st, which):
                w_sb, o_sb = ((wq_sb, qt_sb), (wk_sb, kt_sb))[which]
                ps = mm_ps.tile([128, SQ], F32, tag="mm", name="ps")
                for e in range(NE):
                    nc.tensor.matmul(
                        ps[:],
                        w_sb[:, e, p * 128:(p + 1) * 128],
                        xt_cols(e, st * SQ, (st + 1) * SQ),
                        start=(e == 0), stop=(e == NE - 1),
                    )
                nc.vector.tensor_scalar(
                    o_sb[p][:, st * SQ:(st + 1) * SQ], ps[:],
                    cst_sb[:, which * NPAIR + p:which * NPAIR + p + 1],
                    None, ALU.add,
                )

            # ---- deferred projection units, pulled as PE filler ----
            # unit (deadline, kind, args): deadline = attention step 4*qt+p
            # by which the unit's output is first consumed
            filler = []
            for st in range(4, NKT):
                filler.append((4 * (st // 4), 'v', st))
            for p in range(1, NPAIR):
                for w in range(2):
                    filler.append((p, 'qk', p, 0, w))
            for st in range(1, NQT):
                for p in range(NPAIR):
                    for w in range(2):
                        filler.append((4 * st + p, 'qk', p, st, w))
            filler.sort(key=lambda t: t[0])
            fq = deque(filler)

            def emit_unit(u):
                if u[1] == 'v':
                    emit_v(u[2])
                else:
                    emit_qk_unit(u[2], u[3], u[4])

            def flush_due(s):
                while fq and fq[0][0] <= s:
                    emit_unit(fq.popleft())

            def pull_filler():
                if fq:
                    emit_unit(fq.popleft())

            def attn_kt(qt, p, kt, pv, emit_pv=True):
                nkt_q = 4 * (qt + 1)
                ks = slice(kt * SK, (kt + 1) * SK)
                oi = kt - 4 * qt
                vq = oi * SK if oi > 0 else 0
                sps = st_ps.tile([128, 2 * SQ], F32, tag="st", name="st")
                for u in range(2):
                    nc.tensor.matmul(
                        sps[:, u * SQ + vq:(u + 1) * SQ],
                        kt_sb[p][u * D:(u + 1) * D, ks],
                        qt_sb[p][u * D:(u + 1) * D,
                                 qt * SQ + vq:(qt + 1) * SQ],
                        start=True, stop=True,
                    )
                pt = probs_pool.tile([128, 2 * SQ], BF16, tag="pt", name="pt")
                if oi >= 0:  # diagonal block: exp + mask the valid columns
                    sps3 = sps[:].rearrange("p (u q) -> p u q", u=2)
                    et3 = et_sb[:, oi, :].rearrange("p (u q) -> p u q", u=2)
                    pt3 = pt[:].rearrange("p (u q) -> p u q", u=2)
                    m3 = mask_sb[:, oi, :].rearrange("p (u q) -> p u q", u=2)
                    nc.scalar.activation(et3[:, :, vq:], sps3[:, :, vq:],
                                         AF.Exp, scale=0.125)
                    nc.vector.tensor_tensor(pt3[:, :, vq:], et3[:, :, vq:],
                                            m3[:, :, vq:], ALU.mult)
                else:
                    nc.scalar.activation(pt[:], sps[:], AF.Exp, scale=0.125)
                # keep the PE fed while the exp runs: the filler sits
                # between the scores and PV matmuls in the PE queue;
                # spend units sparingly early so qt3 (most key tiles,
                # no other filler) does not run dry
                if kt % (4 if qt < 3 else 2) == 1:
                    pull_filler()
                if not emit_pv:
                    return pt, vq
                for u in range(2):
                    nc.tensor.matmul(
                        pv[u][0:D + 1, vq:],
                        v_sb[kt][:, 2 * p + u, :],
                        pt[:, u * SQ + vq:(u + 1) * SQ],
                        start=(kt == 0), stop=(kt == nkt_q - 1),
                    )
                return None

            def attn_pv(qt, p, kt, pv, pt, vq):
                nkt_q = 4 * (qt + 1)
                for u in range(2):
                    nc.tensor.matmul(
                        pv[u][0:D + 1, vq:],
                        v_sb[kt][:, 2 * p + u, :],
                        pt[:, u * SQ + vq:(u + 1) * SQ],
                        start=(kt == 0), stop=(kt == nkt_q - 1),
                    )

            def attn_head(qt, p):
                # next pair's first two score+exp tiles, issued before the
                # previous pair's normalization so the exp latency hides
                # behind it (touches only the score/probs rings)
                pv = [pv_ps.tile([128, SQ], F32, tag=f"pv{u}", name=f"pv{u}")
                      for u in range(2)]
                heads = [(kt,) + attn_kt(qt, p, kt, pv, emit_pv=False)
                         for kt in (0, 1)]
                return pv, heads

            def attn_rest(qt, p, pv, heads):
                nkt_q = 4 * (qt + 1)
                for kt, pt, vq in heads:
                    attn_pv(qt, p, kt, pv, pt, vq)
                for kt in range(2, nkt_q):
                    attn_kt(qt, p, kt, pv)
                return pv

            def attn_norm(qt, p, pv):
                # den copies off partition 64 (the custom reciprocal needs a
                # partition-0 input), one reciprocal per head into a shared
                # row, ONE Pool-engine partition_broadcast for the pair, and
                # the multiplies read pv straight from PSUM (the broadcast
                # lives in SBUF, so the one-PSUM-operand rule is satisfied
                # with no staging copies)
                qs = slice(qt * SQ, (qt + 1) * SQ)
                rcp2 = norm_pool.tile([1, 2, SQ], F32, tag="rcp2", name="rcp2")
                for u in range(2):
                    den = norm_pool.tile([1, SQ], F32, tag="den", name="den")
                    nc.vector.tensor_copy(den[:], pv[u][D:D + 1, :])
                    nc.vector.reciprocal_approx_fast(rcp2[:, u, :], den[:])
                rcpb = norm_pool.tile([D, 2, SQ], F32, tag="rcpb", name="rcpb")
                nc.gpsimd.partition_broadcast(rcpb[:], rcp2[:])
                for u in range(2):
                    nc.vector.tensor_tensor(
                        ao_sb[p // 2][u * D:(u + 1) * D, p % 2, qs],
                        pv[u][0:D, :], rcpb[:, u, :], ALU.mult,
                    )

            def emit_op_unit(cq, e, ob):
                # transposed output projection: out^T[e-chunk, tokens]
                ts = slice(cq * CHT, (cq + 1) * CHT)
                ps = mm_ps.tile([128, SQ], F32, tag="mm", name="op")
                for c in range(NPAIR // 2):
                    for i in range(2):
                        nc.tensor.matmul(
                            ps[:, 0:CHT],
                            wo_sb[:, 2 * c + i, e * 128:(e + 1) * 128],
                            ao_sb[c][:, i, ts],
                            start=(c == 0 and i == 0),
                            stop=(c == NPAIR // 2 - 1 and i == 1),
                        )
                nc.vector.tensor_scalar(
                    ob[:, e, :], ps[:, 0:CHT],
                    cst_sb[:, 2 * NPAIR + e:2 * NPAIR + e + 1], None, ALU.add,
                )

            def new_ob():
                return op_sb.tile([128, NE, CHT], BF16, tag="ob", name="ob")

            def emit_rs(cq, w=1):
                nc.gpsimd.collective_compute(
                    "ReduceScatter", ALU.add,
                    ins=[cc_in[cq][:].opt()],
                    outs=[cc_out[cq][:].opt()],
                    replica_groups=REPLICA_GROUPS,
                )
                nc.sync.dma_start(out_d[:, cq * CHT:(cq + w) * CHT],
                                  cc_out[cq][:])

            # ---- schedule ----
            # output units per step: (chunk, [e-list]). Chunks lag their
            # q-tile by one step-group; qt=3's steps pull chunks 4 and 5
            # forward so only chunks 6,7 remain for the tail.
            op_sched = {}
            for qt in (1, 2):
                c0, c1 = 2 * (qt - 1), 2 * (qt - 1) + 1
                op_sched[4 * qt + 0] = [(c0, range(0, 4))]
                op_sched[4 * qt + 1] = [(c0, range(4, 8))]
                op_sched[4 * qt + 2] = [(c1, range(0, 4))]
                op_sched[4 * qt + 3] = [(c1, range(4, 8))]
            op_sched[12] = [(4, range(0, 6))]
            op_sched[13] = [(4, range(6, 8)), (5, range(0, 4))]
            op_sched[14] = [(5, range(4, 6))]
            op_sched[15] = [(5, range(6, 8))]

            for st in range(4):
                emit_v(st)
            emit_qk_unit(0, 0, 0)
            emit_qk_unit(0, 0, 1)
            steps = [(qt, p) for qt in range(NQT) for p in range(NPAIR)]
            nxt = attn_head(0, 0)
            obs = {}
            for qt in range(NQT):
                for p in range(NPAIR):
                    s = 4 * qt + p
                    flush_due(s)
                    pv = attn_rest(qt, p, *nxt)
                    # next step's just-in-time units double as filler over
                    # this pair's normalization chain; its first score+exp
                    # tiles go out before the norm too
                    flush_due(s + 1)
                    nxt = (attn_head(*steps[s + 1])
                           if s + 1 < len(steps) else None)
                    last = s + 1 == len(steps)
                    if last:
                        # norm is DVE/Pool-only; issuing it first lets its
                        # chain overlap the op units and the tail prestart
                        attn_norm(qt, p, pv)
                    for cq, es in op_sched.get(s, []):
                        if cq not in obs:
                            obs[cq] = new_ob()
                        for e in es:
                            emit_op_unit(cq, e, obs[cq])
                        if es[-1] == 7:
                            nc.sync.dma_start(
                                cc_in[cq][:].rearrange("(e p) t -> p e t",
                                                       p=128), obs[cq][:])
                            emit_rs(cq)
                    if not last:
                        attn_norm(qt, p, pv)
            # tail: the last q-tile's two chunks; per-unit DMAs so each RS
            # fires right after its last unit's copy. The first chunk's
            # units pre-start their pair 0-2 matmuls across the attention's
            # now-idle PSUM rings so only the pair-3 matmuls wait for the
            # final pair's norm.
            def op_partial(cq, e, ps, pairs):
                ts = slice(cq * CHT, (cq + 1) * CHT)
                for c, i in pairs:
                    nc.tensor.matmul(
                        ps[:, 0:CHT],
                        wo_sb[:, 2 * c + i, e * 128:(e + 1) * 128],
                        ao_sb[c][:, i, ts],
                        start=(c == 0 and i == 0),
                        stop=(c == 1 and i == 1),
                    )

            cq0 = 2 * (NQT - 1)
            ob = new_ob()
            obs2 = None
            pre = ([(mm_ps, "mm", [128, SQ])] * 2
                   + [(st_ps, "st", [128, 2 * SQ])] * 2
                   + [(pv_ps, "pv0", [128, SQ]), (pv_ps, "pv1", [128, SQ])])
            pss = []
            for e in range(6):
                pool, tag, shape = pre[e]
                ps = pool.tile(shape, F32, tag=tag, name=f"tail{e}")
                op_partial(cq0, e, ps, [(0, 0), (0, 1), (1, 0)])
                pss.append(ps)
            for e in range(6):
                op_partial(cq0, e, pss[e], [(1, 1)])
                nc.vector.tensor_scalar(
                    ob[:, e, :], pss[e][:, 0:CHT],
                    cst_sb[:, 2 * NPAIR + e:2 * NPAIR + e + 1], None, ALU.add)
                nc.sync.dma_start(
                    cc_in[cq0][e * 128:(e + 1) * 128, 0:CHT], ob[:, e, :])
            for e in range(6, NE):
                emit_op_unit(cq0, e, ob)
                nc.sync.dma_start(
                    cc_in[cq0][e * 128:(e + 1) * 128, 0:CHT], ob[:, e, :])
            obs2 = new_ob()
            for e in range(NE):
                emit_op_unit(cq0 + 1, e, obs2)
                nc.sync.dma_start(
                    cc_in[cq0][e * 128:(e + 1) * 128, CHT:2 * CHT],
                    obs2[:, e, :])
            emit_rs(cq0, w=2)

    nc.compile()
    return nc


_NC_CACHE = None


def get_nc():
    global _NC_CACHE
    if _NC_CACHE is None:
        _NC_CACHE = build_kernel()
    return _NC_CACHE


def make_in_maps(X, Wq, Wk, Wv, bq, bk, bv, Wo, bo):
    X = np.asarray(X, np.float32)
    Wq, Wk, Wv = (np.asarray(w, np.float32) for w in (Wq, Wk, Wv))
    bq, bk, bv = (np.asarray(b, np.float32) for b in (bq, bk, bv))
    Wo = np.asarray(Wo, np.float32)
    bo = np.asarray(bo, np.float32)

    m = (np.arange(SQ)[None, :] >=
         (np.arange(4)[:, None, None] * 128 + np.arange(SK)[None, :, None])
         ).astype(BF16_NP)                       # [4, 128, 512]
    masks = np.concatenate([m, m], axis=2)       # [4, 128, 1024] (head pair)
    masks = np.ascontiguousarray(masks.transpose(1, 0, 2))  # [128, 4, 1024]

    def pack(w):  # [1024, n] -> [128, 8, n]
        n = w.shape[1]
        return np.ascontiguousarray(
            w.reshape(NE, 128, n).transpose(1, 0, 2).astype(BF16_NP))

    in_maps = []
    for c in range(N_CORES):
        b, hh = c // 2, c % 2
        hs = slice(hh * HPC, (hh + 1) * HPC)
        wo_c = Wo[hh * HPC * D:(hh + 1) * HPC * D]          # [512, E]
        # fold V-bias and half the output bias into one bias column
        bo2 = 0.5 * bo + bv[hs].reshape(HPC * D) @ wo_c     # [1024]
        bqk = np.concatenate([bq[hs].reshape(NPAIR, 128),
                              bk[hs].reshape(NPAIR, 128)], axis=0)
        in_maps.append({
            "XTa": pack(X[b].T[:, 0:SQ]),
            "XTb": pack(X[b].T[:, SQ:]),
            "Wq": pack(Wq[hs].transpose(1, 0, 2).reshape(E, HPC * D)),
            "Wk": pack(Wk[hs].transpose(1, 0, 2).reshape(E, HPC * D)),
            "Wv": pack(Wv[hs].transpose(1, 0, 2).reshape(E, HPC * D)),
            "Wo": np.ascontiguousarray(
                wo_c.reshape(NPAIR, 128, E).transpose(1, 0, 2)
                .astype(BF16_NP)),                           # [128, 4, 1024]
            "cst": np.ascontiguousarray(
                np.concatenate([bqk, bo2.reshape(NE, 128)], axis=0).T),
            "ones": np.ones((1, 128), np.float32),
            "masks": masks,
        })
    return in_maps


def assemble_output(results):
    out = np.empty((B, S, E), np.float32)
    for b in range(B):
        ev = results[2 * b]["out"].astype(np.float32)      # [512, S] E 0-511
        od = results[2 * b + 1]["out"].astype(np.float32)  # [512, S] E 512+
        out[b, :, 0:E // 2] = ev.T
        out[b, :, E // 2:] = od.T
    return out


def run(in_maps, **kw):
    nc = get_nc()
    return bass_utils.run_bass_kernel_spmd(nc, in_maps,
                                           core_ids=list(range(N_CORES)), **kw)


def kernel(X, Wq, Wk, Wv, bq, bk, bv, Wo, bo):
    in_maps = make_in_maps(X, Wq, Wk, Wv, bq, bk, bv, Wo, bo)
    res = run(in_maps)
    return assemble_output(res.results)


# revision 38
# speedup vs baseline: 1.0092x; 1.0016x over previous
"""Multi-head causal attention (B=4, S=2048, E=1024, H=16, D=64) on 8 TRN2 cores.

Sharding: core c handles batch c//2 and head-half c%2 (8 heads). Each core
computes Q/K/V projections, causal attention for its heads, and a partial
output projection over its heads. Partials are exchanged per 256-token
chunk with a bf16 ReduceScatter over the core pair; the RS splits the
embedding dim, so each core stores one E-half of the final output and the
host transposes/concatenates.

Layouts are transposed (feature-on-partition): the host supplies X^T and
head-packed weights so no on-chip transposes are needed. All inputs are
packed [128, k, n] so each tensor loads with one or two large DMAs.
Attention runs in S^T = K.Q^T layout (keys on partitions); softmax
denominators come from a ones-column appended to V so the PV matmul emits
them for free. Scores for a head pair go side by side into one 2-bank
PSUM tile so a single ScalarE exp covers both; diagonal blocks skip the
causally dead columns in the matmuls, the exp and the mask multiply.

The output projection is emitted transposed (Wo chunk stationary, ao
moving) so the bias lands as a per-partition tensor_scalar on the
PSUM->SBUF copy instead of a K=1 matmul, and the two softmax reciprocal
broadcasts of a head pair share one K=2 matmul.

Scheduling: the exp (ScalarE) paces the attention inner loop, so the PE
needs independent work wherever it would wait. Q/K projections for q-tiles
1-3 and V projections for key-tiles 4-15 are deferred into 8-matmul filler
units pulled between the scores and PV matmuls of the attention loop, and
output-projection units for the previous q-tile run between each pair's
attention and its normalization. This keeps the PE dense enough that the
HAM clock gate stays at full rate.
"""

import sys

sys.path.insert(0, "/opt/trn_rl_repo")

from collections import deque

import numpy as np
import ml_dtypes

import concourse.bass as bass
import concourse.bacc as bacc
import concourse.tile as tile
import concourse.mybir as mybir
import concourse.bass_utils as bass_utils

B, S, E, H, D = 4, 2048, 1024, 16, 64
N_CORES = 8
HPC = H // 2          # heads per core
NPAIR = HPC // 2      # head pairs per core
SQ = 512              # q tile width
SK = 128              # k tile width
NQT = S // SQ         # 4
NKT = S // SK         # 16
NE = E // 128         # 8 contraction tiles
NCH = 8               # output exchange chunks (256 tokens each)
CHT = S // NCH        # 256 tokens per chunk
F32 = mybir.dt.float32
F32R = mybir.dt.float32r
BF16 = mybir.dt.bfloat16
BF16_NP = ml_dtypes.bfloat16

REPLICA_GROUPS = [[0, 1], [2, 3], [4, 5], [6, 7]]
AF = mybir.ActivationFunctionType
ALU = mybir.AluOpType


def build_kernel():
    nc = bacc.Bacc("TRN2", target_bir_lowering=False, debug=False,
                   num_devices=N_CORES)

    xta_d = nc.dram_tensor("XTa", [128, NE, SQ], BF16, kind="ExternalInput")
    xtb_d = nc.dram_tensor("XTb", [128, NE, S - SQ], BF16, kind="ExternalInput")
    wq_d = nc.dram_tensor("Wq", [128, NE, HPC * D], BF16, kind="ExternalInput")
    wk_d = nc.dram_tensor("Wk", [128, NE, HPC * D], BF16, kind="ExternalInput")
    wv_d = nc.dram_tensor("Wv", [128, NE, HPC * D], BF16, kind="ExternalInput")
    wo_d = nc.dram_tensor("Wo", [128, NPAIR, E], BF16, kind="ExternalInput")
    cst_d = nc.dram_tensor("cst", [128, 2 * NPAIR + NE], F32, kind="ExternalInput")
    ones_d = nc.dram_tensor("ones", [1, 128], F32R, kind="ExternalInput")
    mask_d = nc.dram_tensor("masks", [SK, 4, 2 * SQ], BF16, kind="ExternalInput")
    # each core stores its E-half of the output, transposed: [512, S]
    out_d = nc.dram_tensor("out", [E // 2, S], BF16, kind="ExternalOutput")

    with tile.TileContext(nc) as tc:
        with (
            tc.tile_pool(name="persist", bufs=1) as persist,
            tc.tile_pool(name="dram", bufs=1, space="DRAM") as dram,
            tc.tile_pool(name="p1_in", bufs=1) as p1_in,
            tc.tile_pool(name="mm_ps", bufs=2, space="PSUM") as mm_ps,
            tc.tile_pool(name="st_ps", bufs=2, space="PSUM") as st_ps,
            tc.tile_pool(name="pv_ps", bufs=1, space="PSUM") as pv_ps,
            tc.tile_pool(name="probs", bufs=4) as probs_pool,
            tc.tile_pool(name="norm", bufs=2) as norm_pool,
            tc.tile_pool(name="op_sb", bufs=2) as op_sb,
        ):
            # ---- persistent SBUF tensors ----
            qt_sb = [persist.tile([128, S], BF16, tag=f"qt{p}", name=f"qt{p}")
                     for p in range(NPAIR)]
            kt_sb = [persist.tile([128, S], BF16, tag=f"kt{p}", name=f"kt{p}")
                     for p in range(NPAIR)]
            v_sb = [persist.tile([128, HPC, D + 1], BF16, tag=f"v{st}", name=f"v{st}")
                    for st in range(NKT)]
            # ao2[c][:, i, :] holds pair 2c+i (partition = dim within pair)
            ao_sb = [persist.tile([128, 2, S], BF16, tag=f"ao{c}", name=f"ao{c}")
                     for c in range(NPAIR // 2)]
            wo_sb = persist.tile([128, NPAIR, E], BF16, tag="wo")
            cst_sb = persist.tile([128, 2 * NPAIR + NE], F32, tag="cst")
            ones_sb = persist.tile([1, 128], F32R, tag="ones")
            mask_sb = persist.tile([SK, 4, 2 * SQ], BF16, tag="mask")
            et_sb = persist.tile([SK, 4, 2 * SQ], BF16, tag="et")
            cc_in = [dram.tile([E, CHT], BF16, name=f"cc_in{c}")
                     for c in range(NCH - 2)]
            cc_in.append(dram.tile([E, 2 * CHT], BF16, name="cc_in_tail"))
            cc_out = [dram.tile([E // 2, CHT], BF16, name=f"cc_out{c}")
                      for c in range(NCH - 2)]
            cc_out.append(dram.tile([E // 2, 2 * CHT], BF16,
                                    name="cc_out_tail"))

            xta_sb = p1_in.tile([128, NE, SQ], BF16, tag="xta")
            xtb_sb = p1_in.tile([128, NE, S - SQ], BF16, tag="xtb")
            wq_sb = p1_in.tile([128, NE, HPC * D], BF16, tag="wq")
            wk_sb = p1_in.tile([128, NE, HPC * D], BF16, tag="wk")
            wv_sb = p1_in.tile([128, NE, HPC * D], BF16, tag="wv")
            # DMA order = consumption order. Consolidated: one or two large
            # descriptors per tensor so the sync queue issues ~14 DMAs
            # total; the first matmul (V proj) waits only on wv + xt cols
            # 0-511, so those go first and the tiny constants follow the
            # big pre-attention tensors.
            nc.sync.dma_start(wv_sb[:], wv_d[:])
            nc.sync.dma_start(xta_sb[:], xta_d[:])
            nc.sync.dma_start(wq_sb[:], wq_d[:])
            nc.sync.dma_start(wk_sb[:], wk_d[:])
            nc.sync.dma_start(mask_sb[:, 0, :], mask_d[:, 0, :])
            nc.sync.dma_start(cst_sb[:], cst_d[:])
            nc.sync.dma_start(ones_sb[:], ones_d[:])
            for oi in range(1, 4):
                nc.sync.dma_start(mask_sb[:, oi, :], mask_d[:, oi, :])
            nc.sync.dma_start(xtb_sb[:], xtb_d[:])
            nc.sync.dma_start(wo_sb[:], wo_d[:])

            def xt_cols(e, lo, hi):
                if hi <= SQ:
                    return xta_sb[:, e, lo:hi]
                return xtb_sb[:, e, lo - SQ:hi - SQ]

            def emit_v(st):
                ps = mm_ps.tile([128, HPC * D], F32, tag="mm", name="psv")
                for e in range(NE):
                    nc.tensor.matmul(
                        ps[:],
                        xt_cols(e, st * 128, (st + 1) * 128),
                        wv_sb[:, e, :],
                        start=(e == 0), stop=(e == NE - 1),
                    )
                nc.vector.tensor_copy(
                    v_sb[st][:, :, 0:D],
                    ps[:].rearrange("p (h d) -> p h d", h=HPC),
                )
                nc.vector.memset(v_sb[st][:, :, D:D + 1], 1.0)

            def emit_qk_unit(p, st, which):
                w_sb, o_sb = ((wq_sb, qt_sb), (wk_sb, kt_sb))[which]
                ps = mm_ps.tile([128, SQ], F32, tag="mm", name="ps")
                for e in range(NE):
                    nc.tensor.matmul(
                        ps[:],
                        w_sb[:, e, p * 128:(p + 1) * 128],
                        xt_cols(e, st * SQ, (st + 1) * SQ),
                        start=(e == 0), stop=(e == NE - 1),
                    )
                nc.vector.tensor_scalar(
                    o_sb[p][:, st * SQ:(st + 1) * SQ], ps[:],
                    cst_sb[:, which * NPAIR + p:which * NPAIR + p + 1],
                    None, ALU.add,
                )

            # ---- deferred projection units, pulled as PE filler ----
            # unit (deadline, kind, args): deadline = attention step 4*qt+p
            # by which the unit's output is first consumed
            filler = []
            for st in range(4, NKT):
                filler.append((4 * (st // 4), 'v', st))
            for p in range(1, NPAIR):
                for w in range(2):
                    filler.append((p, 'qk', p, 0, w))
            for st in range(1, NQT):
                for p in range(NPAIR):
                    for w in range(2):
                        filler.append((4 * st + p, 'qk', p, st, w))
            filler.sort(key=lambda t: t[0])
            fq = deque(filler)

            def emit_unit(u):
                if u[1] == 'v':
                    emit_v(u[2])
                else:
                    emit_qk_unit(u[2], u[3], u[4])

            def flush_due(s):
                while fq and fq[0][0] <= s:
                    emit_unit(fq.popleft())

            def pull_filler():
                if fq:
                    emit_unit(fq.popleft())

            def attn_kt(qt, p, kt, pv, emit_pv=True):
                nkt_q = 4 * (qt + 1)
                ks = slice(kt * SK, (kt + 1) * SK)
                oi = kt - 4 * qt
                vq = oi * SK if oi > 0 else 0
                sps = st_ps.tile([128, 2 * SQ], F32, tag="st", name="st")
                for u in range(2):
                    nc.tensor.matmul(
                        sps[:, u * SQ + vq:(u + 1) * SQ],
                        kt_sb[p][u * D:(u + 1) * D, ks],
                        qt_sb[p][u * D:(u + 1) * D,
                                 qt * SQ + vq:(qt + 1) * SQ],
                        start=True, stop=True,
                    )
                pt = probs_pool.tile([128, 2 * SQ], BF16, tag="pt", name="pt")
                if oi >= 0:  # diagonal block: exp + mask the valid columns
                    sps3 = sps[:].rearrange("p (u q) -> p u q", u=2)
                    et3 = et_sb[:, oi, :].rearrange("p (u q) -> p u q", u=2)
                    pt3 = pt[:].rearrange("p (u q) -> p u q", u=2)
                    m3 = mask_sb[:, oi, :].rearrange("p (u q) -> p u q", u=2)
                    nc.scalar.activation(et3[:, :, vq:], sps3[:, :, vq:],
                                         AF.Exp, scale=0.125)
                    nc.vector.tensor_tensor(pt3[:, :, vq:], et3[:, :, vq:],
                                            m3[:, :, vq:], ALU.mult)
                else:
                    nc.scalar.activation(pt[:], sps[:], AF.Exp, scale=0.125)
                # keep the PE fed while the exp runs: the filler sits
                # between the scores and PV matmuls in the PE queue;
                # spend units sparingly early so qt3 (most key tiles,
                # no other filler) does not run dry
                if kt % (4 if qt < 3 else 2) == 1:
                    pull_filler()
                if not emit_pv:
                    return pt, vq
                for u in range(2):
                    nc.tensor.matmul(
                        pv[u][0:D + 1, vq:],
                        v_sb[kt][:, 2 * p + u, :],
                        pt[:, u * SQ + vq:(u + 1) * SQ],
                        start=(kt == 0), stop=(kt == nkt_q - 1),
                    )
                return None

            def attn_pv(qt, p, kt, pv, pt, vq):
                nkt_q = 4 * (qt + 1)
                for u in range(2):
                    nc.tensor.matmul(
                        pv[u][0:D + 1, vq:],
                        v_sb[kt][:, 2 * p + u, :],
                        pt[:, u * SQ + vq:(u + 1) * SQ],
                        start=(kt == 0), stop=(kt == nkt_q - 1),
                    )

            def attn_head(qt, p):
                # next pair's first two score+exp tiles, issued before the
                # previous pair's normalization so the exp latency hides
                # behind it (touches only the score/probs rings)
                pv = [pv_ps.tile([128, SQ], F32, tag=f"pv{u}", name=f"pv{u}")
                      for u in range(2)]
                heads = [(kt,) + attn_kt(qt, p, kt, pv, emit_pv=False)
                         for kt in (0, 1)]
                return pv, heads

            def attn_rest(qt, p, pv, heads):
                nkt_q = 4 * (qt + 1)
                for kt, pt, vq in heads:
                    attn_pv(qt, p, kt, pv, pt, vq)
                for kt in range(2, nkt_q):
                    attn_kt(qt, p, kt, pv)
                return pv

            def attn_norm(qt, p, pv):
                # den copies off partition 64 (the custom reciprocal needs a
                # partition-0 input), one reciprocal per head into a shared
                # row, ONE Pool-engine partition_broadcast for the pair, and
                # the multiplies read pv straight from PSUM (the broadcast
                # lives in SBUF, so the one-PSUM-operand rule is satisfied
                # with no staging copies)
                qs = slice(qt * SQ, (qt + 1) * SQ)
                rcp2 = norm_pool.tile([1, 2, SQ], F32, tag="rcp2", name="rcp2")
                for u in range(2):
                    den = norm_pool.tile([1, SQ], F32, tag="den", name="den")
                    nc.vector.tensor_copy(den[:], pv[u][D:D + 1, :])
                    nc.vector.reciprocal_approx_fast(rcp2[:, u, :], den[:])
                rcpb = norm_pool.tile([D, 2, SQ], F32, tag="rcpb", name="rcpb")
                nc.gpsimd.partition_broadcast(rcpb[:], rcp2[:])
                for u in range(2):
                    nc.vector.tensor_tensor(
                        ao_sb[p // 2][u * D:(u + 1) * D, p % 2, qs],
                        pv[u][0:D, :], rcpb[:, u, :], ALU.mult,
                    )

            def emit_op_unit(cq, e, ob):
                # transposed output projection: out^T[e-chunk, tokens]
                ts = slice(cq * CHT, (cq + 1) * CHT)
                ps = mm_ps.tile([128, SQ], F32, tag="mm", name="op")
                for c in range(NPAIR // 2):
                    for i in range(2):
                        nc.tensor.matmul(
                            ps[:, 0:CHT],
                            wo_sb[:, 2 * c + i, e * 128:(e + 1) * 128],
                            ao_sb[c][:, i, ts],
                            start=(c == 0 and i == 0),
                            stop=(c == NPAIR // 2 - 1 and i == 1),
                        )
                nc.vector.tensor_scalar(
                    ob[:, e, :], ps[:, 0:CHT],
                    cst_sb[:, 2 * NPAIR + e:2 * NPAIR + e + 1], None, ALU.add,
                )

            def new_ob():
                return op_sb.tile([128, NE, CHT], BF16, tag="ob", name="ob")

            def emit_rs(cq, w=1):
                nc.gpsimd.collective_compute(
                    "ReduceScatter", ALU.add,
                    ins=[cc_in[cq][:].opt()],
                    outs=[cc_out[cq][:].opt()],
                    replica_groups=REPLICA_GROUPS,
                )
                nc.sync.dma_start(out_d[:, cq * CHT:(cq + w) * CHT],
                                  cc_out[cq][:])

            # ---- schedule ----
            # output units per step: (chunk, [e-list]). Chunks lag their
            # q-tile by one step-group; qt=3's steps pull chunks 4 and 5
            # forward so only chunks 6,7 remain for the tail.
            op_sched = {}
            for qt in (1, 2):
                c0, c1 = 2 * (qt - 1), 2 * (qt - 1) + 1
                op_sched[4 * qt + 0] = [(c0, range(0, 4))]
                op_sched[4 * qt + 1] = [(c0, range(4, 8))]
                op_sched[4 * qt + 2] = [(c1, range(0, 4))]
                op_sched[4 * qt + 3] = [(c1, range(4, 8))]
            op_sched[12] = [(4, range(0, 4))]
            op_sched[13] = [(4, range(4, 8)), (5, range(0, 2))]
            op_sched[14] = [(5, range(2, 8))]
            op_sched[15] = []

            for st in range(4):
                emit_v(st)
            emit_qk_unit(0, 0, 0)
            emit_qk_unit(0, 0, 1)
            steps = [(qt, p) for qt in range(NQT) for p in range(NPAIR)]
            nxt = attn_head(0, 0)
            obs = {}
            for qt in range(NQT):
                for p in range(NPAIR):
                    s = 4 * qt + p
                    flush_due(s)
                    pv = attn_rest(qt, p, *nxt)
                    # next step's just-in-time units double as filler over
                    # this pair's normalization chain; its first score+exp
                    # tiles go out before the norm too
                    flush_due(s + 1)
                    nxt = (attn_head(*steps[s + 1])
                           if s + 1 < len(steps) else None)
                    last = s + 1 == len(steps)
                    if last:
                        # norm is DVE/Pool-only; issuing it first lets its
                        # chain overlap the op units and the tail prestart
                        attn_norm(qt, p, pv)
                    for cq, es in op_sched.get(s, []):
                        if cq not in obs:
                            obs[cq] = new_ob()
                        for e in es:
                            emit_op_unit(cq, e, obs[cq])
                        if es[-1] == 7:
                            nc.sync.dma_start(
                                cc_in[cq][:].rearrange("(e p) t -> p e t",
                                                       p=128), obs[cq][:])
                            emit_rs(cq)
                    if not last:
                        attn_norm(qt, p, pv)
            # tail: the last q-tile's two chunks; per-unit DMAs so each RS
            # fires right after its last unit's copy. The first chunk's
            # units pre-start their pair 0-2 matmuls across the attention's
            # now-idle PSUM rings so only the pair-3 matmuls wait for the
            # final pair's norm.
            def op_partial(cq, e, ps, pairs):
                ts = slice(cq * CHT, (cq + 1) * CHT)
                for c, i in pairs:
                    nc.tensor.matmul(
                        ps[:, 0:CHT],
                        wo_sb[:, 2 * c + i, e * 128:(e + 1) * 128],
                        ao_sb[c][:, i, ts],
                        start=(c == 0 and i == 0),
                        stop=(c == 1 and i == 1),
                    )

            cq0 = 2 * (NQT - 1)
            ob = new_ob()
            obs2 = None
            pre = ([(mm_ps, "mm", [128, SQ])] * 2
                   + [(st_ps, "st", [128, 2 * SQ])] * 2
                   + [(pv_ps, "pv0", [128, SQ]), (pv_ps, "pv1", [128, SQ])])
            pss = []
            for e in range(6):
                pool, tag, shape = pre[e]
                ps = pool.tile(shape, F32, tag=tag, name=f"tail{e}")
                op_partial(cq0, e, ps, [(0, 0), (0, 1), (1, 0)])
                pss.append(ps)
            for e in range(6):
                op_partial(cq0, e, pss[e], [(1, 1)])
                nc.vector.tensor_scalar(
                    ob[:, e, :], pss[e][:, 0:CHT],
                    cst_sb[:, 2 * NPAIR + e:2 * NPAIR + e + 1], None, ALU.add)
            for e in range(6, NE):
                emit_op_unit(cq0, e, ob)
            nc.sync.dma_start(
                cc_in[cq0][:, 0:CHT].rearrange("(e p) t -> p e t", p=128),
                ob[:])
            obs2 = new_ob()
            for e in range(NE):
                emit_op_unit(cq0 + 1, e, obs2)
            nc.sync.dma_start(
                cc_in[cq0][:, CHT:2 * CHT].rearrange("(e p) t -> p e t",
                                                     p=128), obs2[:])
            emit_rs(cq0, w=2)

    nc.compile()
    return nc


_NC_CACHE = None


def get_nc():
    global _NC_CACHE
    if _NC_CACHE is None:
        _NC_CACHE = build_kernel()
    return _NC_CACHE


def make_in_maps(X, Wq, Wk, Wv, bq, bk, bv, Wo, bo):
    X = np.asarray(X, np.float32)
    Wq, Wk, Wv = (np.asarray(w, np.float32) for w in (Wq, Wk, Wv))
    bq, bk, bv = (np.asarray(b, np.float32) for b in (bq, bk, bv))
    Wo = np.asarray(Wo, np.float32)
    bo = np.asarray(bo, np.float32)

    m = (np.arange(SQ)[None, :] >=
         (np.arange(4)[:, None, None] * 128 + np.arange(SK)[None, :, None])
         ).astype(BF16_NP)                       # [4, 128, 512]
    masks = np.concatenate([m, m], axis=2)       # [4, 128, 1024] (head pair)
    masks = np.ascontiguousarray(masks.transpose(1, 0, 2))  # [128, 4, 1024]

    def pack(w):  # [1024, n] -> [128, 8, n]
        n = w.shape[1]
        return np.ascontiguousarray(
            w.reshape(NE, 128, n).transpose(1, 0, 2).astype(BF16_NP))

    in_maps = []
    for c in range(N_CORES):
        b, hh = c // 2, c % 2
        hs = slice(hh * HPC, (hh + 1) * HPC)
        wo_c = Wo[hh * HPC * D:(hh + 1) * HPC * D]          # [512, E]
        # fold V-bias and half the output bias into one bias column
        bo2 = 0.5 * bo + bv[hs].reshape(HPC * D) @ wo_c     # [1024]
        bqk = np.concatenate([bq[hs].reshape(NPAIR, 128),
                              bk[hs].reshape(NPAIR, 128)], axis=0)
        in_maps.append({
            "XTa": pack(X[b].T[:, 0:SQ]),
            "XTb": pack(X[b].T[:, SQ:]),
            "Wq": pack(Wq[hs].transpose(1, 0, 2).reshape(E, HPC * D)),
            "Wk": pack(Wk[hs].transpose(1, 0, 2).reshape(E, HPC * D)),
            "Wv": pack(Wv[hs].transpose(1, 0, 2).reshape(E, HPC * D)),
            "Wo": np.ascontiguousarray(
                wo_c.reshape(NPAIR, 128, E).transpose(1, 0, 2)
                .astype(BF16_NP)),                           # [128, 4, 1024]
            "cst": np.ascontiguousarray(
                np.concatenate([bqk, bo2.reshape(NE, 128)], axis=0).T),
            "ones": np.ones((1, 128), np.float32),
            "masks": masks,
        })
    return in_maps


def assemble_output(results):
    out = np.empty((B, S, E), np.float32)
    for b in range(B):
        ev = results[2 * b]["out"].astype(np.float32)      # [512, S] E 0-511
        od = results[2 * b + 1]["out"].astype(np.float32)  # [512, S] E 512+
        out[b, :, 0:E // 2] = ev.T
        out[b, :, E // 2:] = od.T
    return out


def run(in_maps, **kw):
    nc = get_nc()
    return bass_utils.run_bass_kernel_spmd(nc, in_maps,
                                           core_ids=list(range(N_CORES)), **kw)


def kernel(X, Wq, Wk, Wv, bq, bk, bv, Wo, bo):
    in_maps = make_in_maps(X, Wq, Wk, Wv, bq, bk, bv, Wo, bo)
    res = run(in_maps)
    return assemble_output(res.results)


# revision 39
# speedup vs baseline: 1.0384x; 1.0290x over previous
"""Multi-head causal attention (B=4, S=2048, E=1024, H=16, D=64) on 8 TRN2 cores.

Sharding: core c handles batch c//2 and head-half c%2 (8 heads). Each core
computes Q/K/V projections, causal attention for its heads, and a partial
output projection over its heads. Partials are exchanged per 256-token
chunk with a bf16 ReduceScatter over the core pair; the RS splits the
embedding dim, so each core stores one E-half of the final output and the
host transposes/concatenates.

Layouts are transposed (feature-on-partition): the host supplies X^T and
head-packed weights so no on-chip transposes are needed. All inputs are
packed [128, k, n] so each tensor loads with one or two large DMAs.
Attention runs in S^T = K.Q^T layout (keys on partitions); softmax
denominators come from a ones-column appended to V so the PV matmul emits
them for free. Scores for a head pair go side by side into one 2-bank
PSUM tile so a single ScalarE exp covers both; diagonal blocks skip the
causally dead columns in the matmuls, the exp and the mask multiply.

The output projection is emitted transposed (Wo chunk stationary, ao
moving) so the bias lands as a per-partition tensor_scalar on the
PSUM->SBUF copy instead of a K=1 matmul, and the two softmax reciprocal
broadcasts of a head pair share one K=2 matmul.

Scheduling: the exp (ScalarE) paces the attention inner loop, so the PE
needs independent work wherever it would wait. Q/K projections for q-tiles
1-3 and V projections for key-tiles 4-15 are deferred into 8-matmul filler
units pulled between the scores and PV matmuls of the attention loop, and
output-projection units for the previous q-tile run between each pair's
attention and its normalization. This keeps the PE dense enough that the
HAM clock gate stays at full rate.
"""

import sys

sys.path.insert(0, "/opt/trn_rl_repo")

from collections import deque

import numpy as np
import ml_dtypes

import concourse.bass as bass
import concourse.bacc as bacc
import concourse.tile as tile
import concourse.mybir as mybir
import concourse.bass_utils as bass_utils

B, S, E, H, D = 4, 2048, 1024, 16, 64
N_CORES = 8
HPC = H // 2          # heads per core
NPAIR = HPC // 2      # head pairs per core
SQ = 512              # q tile width
SK = 128              # k tile width
NQT = S // SQ         # 4
NKT = S // SK         # 16
NE = E // 128         # 8 contraction tiles
NCH = 8               # output exchange chunks (256 tokens each)
CHT = S // NCH        # 256 tokens per chunk
F32 = mybir.dt.float32
F32R = mybir.dt.float32r
BF16 = mybir.dt.bfloat16
BF16_NP = ml_dtypes.bfloat16

REPLICA_GROUPS = [[0, 1], [2, 3], [4, 5], [6, 7]]
AF = mybir.ActivationFunctionType
ALU = mybir.AluOpType


def build_kernel():
    nc = bacc.Bacc("TRN2", target_bir_lowering=False, debug=False,
                   num_devices=N_CORES)

    xta_d = nc.dram_tensor("XTa", [128, NE, SQ], BF16, kind="ExternalInput")
    xtb_d = nc.dram_tensor("XTb", [128, NE, S - SQ], BF16, kind="ExternalInput")
    wq_d = nc.dram_tensor("Wq", [128, NE, HPC * D], BF16, kind="ExternalInput")
    wk_d = nc.dram_tensor("Wk", [128, NE, HPC * D], BF16, kind="ExternalInput")
    wv_d = nc.dram_tensor("Wv", [128, NE, HPC * D], BF16, kind="ExternalInput")
    wo_d = nc.dram_tensor("Wo", [128, NPAIR, E], BF16, kind="ExternalInput")
    cst_d = nc.dram_tensor("cst", [128, 2 * NPAIR + NE], F32, kind="ExternalInput")
    ones_d = nc.dram_tensor("ones", [1, 128], F32R, kind="ExternalInput")
    mask_d = nc.dram_tensor("masks", [SK, 4, 2 * SQ], BF16, kind="ExternalInput")
    # each core stores its E-half of the output, transposed: [512, S]
    out_d = nc.dram_tensor("out", [E // 2, S], BF16, kind="ExternalOutput")

    with tile.TileContext(nc) as tc:
        with (
            tc.tile_pool(name="persist", bufs=1) as persist,
            tc.tile_pool(name="dram", bufs=1, space="DRAM") as dram,
            tc.tile_pool(name="p1_in", bufs=1) as p1_in,
            tc.tile_pool(name="mm_ps", bufs=2, space="PSUM") as mm_ps,
            tc.tile_pool(name="st_ps", bufs=2, space="PSUM") as st_ps,
            tc.tile_pool(name="pv_ps", bufs=1, space="PSUM") as pv_ps,
            tc.tile_pool(name="probs", bufs=4) as probs_pool,
            tc.tile_pool(name="norm", bufs=2) as norm_pool,
            tc.tile_pool(name="op_sb", bufs=2) as op_sb,
        ):
            # ---- persistent SBUF tensors ----
            qt_sb = [persist.tile([128, S], BF16, tag=f"qt{p}", name=f"qt{p}")
                     for p in range(NPAIR)]
            kt_sb = [persist.tile([128, S], BF16, tag=f"kt{p}", name=f"kt{p}")
                     for p in range(NPAIR)]
            v_sb = [persist.tile([128, HPC, D + 1], BF16, tag=f"v{st}", name=f"v{st}")
                    for st in range(NKT)]
            # ao2[c][:, i, :] holds pair 2c+i (partition = dim within pair)
            ao_sb = [persist.tile([128, 2, S], BF16, tag=f"ao{c}", name=f"ao{c}")
                     for c in range(NPAIR // 2)]
            wo_sb = persist.tile([128, NPAIR, E], BF16, tag="wo")
            cst_sb = persist.tile([128, 2 * NPAIR + NE], F32, tag="cst")
            ones_sb = persist.tile([1, 128], F32R, tag="ones")
            mask_sb = persist.tile([SK, 4, 2 * SQ], BF16, tag="mask")
            et_sb = persist.tile([SK, 4, 2 * SQ], BF16, tag="et")
            cc_in = [dram.tile([E, CHT], BF16, name=f"cc_in{c}")
                     for c in range(NCH - 2)]
            cc_in.append(dram.tile([E, 2 * CHT], BF16, name="cc_in_tail"))
            cc_out = [dram.tile([E // 2, CHT], BF16, name=f"cc_out{c}")
                      for c in range(NCH - 2)]
            cc_out.append(dram.tile([E // 2, 2 * CHT], BF16,
                                    name="cc_out_tail"))

            xta_sb = p1_in.tile([128, NE, SQ], BF16, tag="xta")
            xtb_sb = p1_in.tile([128, NE, S - SQ], BF16, tag="xtb")
            wq_sb = p1_in.tile([128, NE, HPC * D], BF16, tag="wq")
            wk_sb = p1_in.tile([128, NE, HPC * D], BF16, tag="wk")
            wv_sb = p1_in.tile([128, NE, HPC * D], BF16, tag="wv")
            # DMA order = consumption order. Consolidated: one or two large
            # descriptors per tensor so the sync queue issues ~14 DMAs
            # total; the first matmul (V proj) waits only on wv + xt cols
            # 0-511, so those go first and the tiny constants follow the
            # big pre-attention tensors.
            nc.sync.dma_start(wv_sb[:], wv_d[:])
            nc.sync.dma_start(xta_sb[:], xta_d[:])
            nc.sync.dma_start(wq_sb[:], wq_d[:])
            nc.sync.dma_start(wk_sb[:], wk_d[:])
            nc.sync.dma_start(mask_sb[:, 0, :], mask_d[:, 0, :])
            nc.sync.dma_start(cst_sb[:], cst_d[:])
            nc.sync.dma_start(ones_sb[:], ones_d[:])
            for oi in range(1, 4):
                nc.sync.dma_start(mask_sb[:, oi, :], mask_d[:, oi, :])
            nc.sync.dma_start(xtb_sb[:], xtb_d[:])
            nc.sync.dma_start(wo_sb[:], wo_d[:])

            def xt_cols(e, lo, hi):
                if hi <= SQ:
                    return xta_sb[:, e, lo:hi]
                return xtb_sb[:, e, lo - SQ:hi - SQ]

            def emit_v(st):
                ps = mm_ps.tile([128, HPC * D], F32, tag="mm", name="psv")
                for e in range(NE):
                    nc.tensor.matmul(
                        ps[:],
                        xt_cols(e, st * 128, (st + 1) * 128),
                        wv_sb[:, e, :],
                        start=(e == 0), stop=(e == NE - 1),
                    )
                nc.vector.tensor_copy(
                    v_sb[st][:, :, 0:D],
                    ps[:].rearrange("p (h d) -> p h d", h=HPC),
                )
                nc.vector.memset(v_sb[st][:, :, D:D + 1], 1.0)

            def emit_qk_unit(p, st, which):
                w_sb, o_sb = ((wq_sb, qt_sb), (wk_sb, kt_sb))[which]
                ps = mm_ps.tile([128, SQ], F32, tag="mm", name="ps")
                for e in range(NE):
                    nc.tensor.matmul(
                        ps[:],
                        w_sb[:, e, p * 128:(p + 1) * 128],
                        xt_cols(e, st * SQ, (st + 1) * SQ),
                        start=(e == 0), stop=(e == NE - 1),
                    )
                nc.vector.tensor_scalar(
                    o_sb[p][:, st * SQ:(st + 1) * SQ], ps[:],
                    cst_sb[:, which * NPAIR + p:which * NPAIR + p + 1],
                    None, ALU.add,
                )

            # ---- deferred projection units, pulled as PE filler ----
            # unit (deadline, kind, args): deadline = attention step 4*qt+p
            # by which the unit's output is first consumed
            filler = []
            for st in range(4, NKT):
                filler.append((4 * (st // 4), 'v', st))
            for p in range(1, NPAIR):
                for w in range(2):
                    filler.append((p, 'qk', p, 0, w))
            for st in range(1, NQT):
                for p in range(NPAIR):
                    for w in range(2):
                        filler.append((4 * st + p, 'qk', p, st, w))
            filler.sort(key=lambda t: t[0])
            fq = deque(filler)

            def emit_unit(u):
                if u[1] == 'v':
                    emit_v(u[2])
                else:
                    emit_qk_unit(u[2], u[3], u[4])

            def flush_due(s):
                while fq and fq[0][0] <= s:
                    emit_unit(fq.popleft())

            def pull_filler():
                if fq:
                    emit_unit(fq.popleft())

            def attn_kt(qt, p, kt, pv, emit_pv=True):
                nkt_q = 4 * (qt + 1)
                ks = slice(kt * SK, (kt + 1) * SK)
                oi = kt - 4 * qt
                vq = oi * SK if oi > 0 else 0
                sps = st_ps.tile([128, 2 * SQ], F32, tag="st", name="st")
                for u in range(2):
                    nc.tensor.matmul(
                        sps[:, u * SQ + vq:(u + 1) * SQ],
                        kt_sb[p][u * D:(u + 1) * D, ks],
                        qt_sb[p][u * D:(u + 1) * D,
                                 qt * SQ + vq:(qt + 1) * SQ],
                        start=True, stop=True,
                    )
                pt = probs_pool.tile([128, 2 * SQ], BF16, tag="pt", name="pt")
                if oi >= 0:  # diagonal block: exp + mask the valid columns
                    sps3 = sps[:].rearrange("p (u q) -> p u q", u=2)
                    et3 = et_sb[:, oi, :].rearrange("p (u q) -> p u q", u=2)
                    pt3 = pt[:].rearrange("p (u q) -> p u q", u=2)
                    m3 = mask_sb[:, oi, :].rearrange("p (u q) -> p u q", u=2)
                    nc.scalar.activation(et3[:, :, vq:], sps3[:, :, vq:],
                                         AF.Exp, scale=0.125)
                    nc.vector.tensor_tensor(pt3[:, :, vq:], et3[:, :, vq:],
                                            m3[:, :, vq:], ALU.mult)
                else:
                    nc.scalar.activation(pt[:], sps[:], AF.Exp, scale=0.125)
                # keep the PE fed while the exp runs: the filler sits
                # between the scores and PV matmuls in the PE queue;
                # spend units sparingly early so qt3 (most key tiles,
                # no other filler) does not run dry
                if kt % (4 if qt < 3 else 2) == 1:
                    pull_filler()
                if not emit_pv:
                    return pt, vq
                for u in range(2):
                    nc.tensor.matmul(
                        pv[u][0:D + 1, vq:],
                        v_sb[kt][:, 2 * p + u, :],
                        pt[:, u * SQ + vq:(u + 1) * SQ],
                        start=(kt == 0), stop=(kt == nkt_q - 1),
                    )
                return None

            def attn_pv(qt, p, kt, pv, pt, vq):
                nkt_q = 4 * (qt + 1)
                for u in range(2):
                    nc.tensor.matmul(
                        pv[u][0:D + 1, vq:],
                        v_sb[kt][:, 2 * p + u, :],
                        pt[:, u * SQ + vq:(u + 1) * SQ],
                        start=(kt == 0), stop=(kt == nkt_q - 1),
                    )

            def attn_head(qt, p):
                # next pair's first two score+exp tiles, issued before the
                # previous pair's normalization so the exp latency hides
                # behind it (touches only the score/probs rings)
                pv = [pv_ps.tile([128, SQ], F32, tag=f"pv{u}", name=f"pv{u}")
                      for u in range(2)]
                heads = [(kt,) + attn_kt(qt, p, kt, pv, emit_pv=False)
                         for kt in (0, 1)]
                return pv, heads

            def attn_rest(qt, p, pv, heads):
                nkt_q = 4 * (qt + 1)
                for kt, pt, vq in heads:
                    attn_pv(qt, p, kt, pv, pt, vq)
                for kt in range(2, nkt_q):
                    attn_kt(qt, p, kt, pv)
                return pv

            def attn_norm(qt, p, pv):
                # den copies off partition 64 (the custom reciprocal needs a
                # partition-0 input), one reciprocal per head into a shared
                # row, ONE Pool-engine partition_broadcast for the pair, and
                # the multiplies read pv straight from PSUM (the broadcast
                # lives in SBUF, so the one-PSUM-operand rule is satisfied
                # with no staging copies)
                qs = slice(qt * SQ, (qt + 1) * SQ)
                rcp2 = norm_pool.tile([1, 2, SQ], F32, tag="rcp2", name="rcp2")
                dens = []
                for u in range(2):
                    den = norm_pool.tile([1, SQ], F32, tag="den", name="den")
                    nc.vector.tensor_copy(den[:], pv[u][D:D + 1, :])
                    dens.append(den)
                # pv copies straight after the dens: they are the last pv
                # readers, so the pv PSUM ring frees for the next pair's PV
                # ~3us earlier than if the mults read pv directly
                pvs = [norm_pool.tile([D, SQ], F32, tag=f"pvs{u}",
                                      name=f"pvs{u}") for u in range(2)]
                for u in range(2):
                    nc.vector.tensor_copy(pvs[u][:], pv[u][0:D, :])
                for u in range(2):
                    nc.vector.reciprocal_approx_fast(rcp2[:, u, :], dens[u][:])
                rcpb = norm_pool.tile([D, 2, SQ], F32, tag="rcpb", name="rcpb")
                nc.gpsimd.partition_broadcast(rcpb[:], rcp2[:])
                for u in range(2):
                    nc.vector.tensor_tensor(
                        ao_sb[p // 2][u * D:(u + 1) * D, p % 2, qs],
                        pvs[u][:], rcpb[:, u, :], ALU.mult,
                    )

            def emit_op_unit(cq, e, ob):
                # transposed output projection: out^T[e-chunk, tokens]
                ts = slice(cq * CHT, (cq + 1) * CHT)
                ps = mm_ps.tile([128, SQ], F32, tag="mm", name="op")
                for c in range(NPAIR // 2):
                    for i in range(2):
                        nc.tensor.matmul(
                            ps[:, 0:CHT],
                            wo_sb[:, 2 * c + i, e * 128:(e + 1) * 128],
                            ao_sb[c][:, i, ts],
                            start=(c == 0 and i == 0),
                            stop=(c == NPAIR // 2 - 1 and i == 1),
                        )
                nc.vector.tensor_scalar(
                    ob[:, e, :], ps[:, 0:CHT],
                    cst_sb[:, 2 * NPAIR + e:2 * NPAIR + e + 1], None, ALU.add,
                )

            def new_ob():
                return op_sb.tile([128, NE, CHT], BF16, tag="ob", name="ob")

            def emit_rs(cq, w=1):
                nc.gpsimd.collective_compute(
                    "ReduceScatter", ALU.add,
                    ins=[cc_in[cq][:].opt()],
                    outs=[cc_out[cq][:].opt()],
                    replica_groups=REPLICA_GROUPS,
                )
                nc.sync.dma_start(out_d[:, cq * CHT:(cq + w) * CHT],
                                  cc_out[cq][:])

            # ---- schedule ----
            # output units per step: (chunk, [e-list]). Chunks lag their
            # q-tile by one step-group; qt=3's steps pull chunks 4 and 5
            # forward so only chunks 6,7 remain for the tail.
            op_sched = {}
            for qt in (1, 2):
                c0, c1 = 2 * (qt - 1), 2 * (qt - 1) + 1
                op_sched[4 * qt + 0] = [(c0, range(0, 4))]
                op_sched[4 * qt + 1] = [(c0, range(4, 8))]
                op_sched[4 * qt + 2] = [(c1, range(0, 4))]
                op_sched[4 * qt + 3] = [(c1, range(4, 8))]
            op_sched[12] = [(4, range(0, 4))]
            op_sched[13] = [(4, range(4, 8)), (5, range(0, 2))]
            op_sched[14] = [(5, range(2, 8))]
            op_sched[15] = []

            for st in range(4):
                emit_v(st)
            emit_qk_unit(0, 0, 0)
            emit_qk_unit(0, 0, 1)
            steps = [(qt, p) for qt in range(NQT) for p in range(NPAIR)]
            nxt = attn_head(0, 0)
            obs = {}
            for qt in range(NQT):
                for p in range(NPAIR):
                    s = 4 * qt + p
                    flush_due(s)
                    pv = attn_rest(qt, p, *nxt)
                    # next step's just-in-time units double as filler over
                    # this pair's normalization chain; its first score+exp
                    # tiles go out before the norm too
                    flush_due(s + 1)
                    nxt = (attn_head(*steps[s + 1])
                           if s + 1 < len(steps) else None)
                    last = s + 1 == len(steps)
                    if last:
                        # norm is DVE/Pool-only; issuing it first lets its
                        # chain overlap the op units and the tail prestart
                        attn_norm(qt, p, pv)
                    for cq, es in op_sched.get(s, []):
                        if cq not in obs:
                            obs[cq] = new_ob()
                        for e in es:
                            emit_op_unit(cq, e, obs[cq])
                        if es[-1] == 7:
                            nc.sync.dma_start(
                                cc_in[cq][:].rearrange("(e p) t -> p e t",
                                                       p=128), obs[cq][:])
                            emit_rs(cq)
                    if not last:
                        attn_norm(qt, p, pv)
            # tail: the last q-tile's two chunks; per-unit DMAs so each RS
            # fires right after its last unit's copy. The first chunk's
            # units pre-start their pair 0-2 matmuls across the attention's
            # now-idle PSUM rings so only the pair-3 matmuls wait for the
            # final pair's norm.
            def op_partial(cq, e, ps, pairs):
                ts = slice(cq * CHT, (cq + 1) * CHT)
                for c, i in pairs:
                    nc.tensor.matmul(
                        ps[:, 0:CHT],
                        wo_sb[:, 2 * c + i, e * 128:(e + 1) * 128],
                        ao_sb[c][:, i, ts],
                        start=(c == 0 and i == 0),
                        stop=(c == 1 and i == 1),
                    )

            cq0 = 2 * (NQT - 1)
            ob = new_ob()
            obs2 = None
            pre = ([(mm_ps, "mm", [128, SQ])] * 2
                   + [(st_ps, "st", [128, 2 * SQ])] * 2
                   + [(pv_ps, "pv0", [128, SQ]), (pv_ps, "pv1", [128, SQ])])
            pss = []
            for e in range(6):
                pool, tag, shape = pre[e]
                ps = pool.tile(shape, F32, tag=tag, name=f"tail{e}")
                op_partial(cq0, e, ps, [(0, 0), (0, 1), (1, 0)])
                pss.append(ps)
            for e in range(6):
                op_partial(cq0, e, pss[e], [(1, 1)])
                nc.vector.tensor_scalar(
                    ob[:, e, :], pss[e][:, 0:CHT],
                    cst_sb[:, 2 * NPAIR + e:2 * NPAIR + e + 1], None, ALU.add)
            for e in range(6, NE):
                emit_op_unit(cq0, e, ob)
            nc.sync.dma_start(
                cc_in[cq0][:, 0:CHT].rearrange("(e p) t -> p e t", p=128),
                ob[:])
            obs2 = new_ob()
            for e in range(NE):
                emit_op_unit(cq0 + 1, e, obs2)
            nc.sync.dma_start(
                cc_in[cq0][:, CHT:2 * CHT].rearrange("(e p) t -> p e t",
                                                     p=128), obs2[:])
            emit_rs(cq0, w=2)

    nc.compile()
    return nc


_NC_CACHE = None


def get_nc():
    global _NC_CACHE
    if _NC_CACHE is None:
        _NC_CACHE = build_kernel()
    return _NC_CACHE


def make_in_maps(X, Wq, Wk, Wv, bq, bk, bv, Wo, bo):
    X = np.asarray(X, np.float32)
    Wq, Wk, Wv = (np.asarray(w, np.float32) for w in (Wq, Wk, Wv))
    bq, bk, bv = (np.asarray(b, np.float32) for b in (bq, bk, bv))
    Wo = np.asarray(Wo, np.float32)
    bo = np.asarray(bo, np.float32)

    m = (np.arange(SQ)[None, :] >=
         (np.arange(4)[:, None, None] * 128 + np.arange(SK)[None, :, None])
         ).astype(BF16_NP)                       # [4, 128, 512]
    masks = np.concatenate([m, m], axis=2)       # [4, 128, 1024] (head pair)
    masks = np.ascontiguousarray(masks.transpose(1, 0, 2))  # [128, 4, 1024]

    def pack(w):  # [1024, n] -> [128, 8, n]
        n = w.shape[1]
        return np.ascontiguousarray(
            w.reshape(NE, 128, n).transpose(1, 0, 2).astype(BF16_NP))

    in_maps = []
    for c in range(N_CORES):
        b, hh = c // 2, c % 2
        hs = slice(hh * HPC, (hh + 1) * HPC)
        wo_c = Wo[hh * HPC * D:(hh + 1) * HPC * D]          # [512, E]
        # fold V-bias and half the output bias into one bias column
        bo2 = 0.5 * bo + bv[hs].reshape(HPC * D) @ wo_c     # [1024]
        bqk = np.concatenate([bq[hs].reshape(NPAIR, 128),
                              bk[hs].reshape(NPAIR, 128)], axis=0)
        in_maps.append({
            "XTa": pack(X[b].T[:, 0:SQ]),
            "XTb": pack(X[b].T[:, SQ:]),
            "Wq": pack(Wq[hs].transpose(1, 0, 2).reshape(E, HPC * D)),
            "Wk": pack(Wk[hs].transpose(1, 0, 2).reshape(E, HPC * D)),
            "Wv": pack(Wv[hs].transpose(1, 0, 2).reshape(E, HPC * D)),
            "Wo": np.ascontiguousarray(
                wo_c.reshape(NPAIR, 128, E).transpose(1, 0, 2)
                .astype(BF16_NP)),                           # [128, 4, 1024]
            "cst": np.ascontiguousarray(
                np.concatenate([bqk, bo2.reshape(NE, 128)], axis=0).T),
            "ones": np.ones((1, 128), np.float32),
            "masks": masks,
        })
    return in_maps


def assemble_output(results):
    out = np.empty((B, S, E), np.float32)
    for b in range(B):
        ev = results[2 * b]["out"].astype(np.float32)      # [512, S] E 0-511
        od = results[2 * b + 1]["out"].astype(np.float32)  # [512, S] E 512+
        out[b, :, 0:E // 2] = ev.T
        out[b, :, E // 2:] = od.T
    return out


def run(in_maps, **kw):
    nc = get_nc()
    return bass_utils.run_bass_kernel_spmd(nc, in_maps,
                                           core_ids=list(range(N_CORES)), **kw)


def kernel(X, Wq, Wk, Wv, bq, bk, bv, Wo, bo):
    in_maps = make_in_maps(X, Wq, Wk, Wv, bq, bk, bv, Wo, bo)
    res = run(in_maps)
    return assemble_output(res.results)
